# revision 6
# baseline (speedup 1.0000x reference)
"""Causal multi-head attention (B=8, S=1024, D=768, H=12, Dh=64) on 8 TRN2
NeuronCores, batch-parallel (one batch element per core).

Per-core Bass/Tile kernel, structured for engine overlap:
  - x DMAs ride the SP HWDGE ring while W DMAs ride the ACT ring in parallel.
  - Per s-chunk: PE transposes x -> x^T (bf16) in batches of 3 per PSUM bank
    (one DVE copy per batch), then immediately projects V chunks so PE starts
    ~2us into the kernel.
  - Per head-pair group g: Q^T/K^T projections (weight-pair stationary, x^T
    moving) accumulate into a single 2-bank [P,1024] PSUM tile (one DVE copy),
    then attention for the two heads — ScalarE exp work of group g overlaps
    the PE projection work of group g+1.
  - Scores are computed transposed S^T[t, s] = K·Q^T with causal skip into a
    2-bank [P,1024] PSUM tile; ONE exp per key-chunk on ScalarE (scale=1/8
    folded in, no max subtraction — scores are O(5)); diagonal block masked by
    a 0/1 triangle multiply on GpSimd (keeps DVE free).
  - V' carries a ones-column per head so the PV matmul accumulates softmax
    denominators in ctx^T row 64; per 4 s-chunks: 4 PE transposes into one
    PSUM bank + one strided reciprocal + one broadcast multiply normalize
    straight into the output layout.
"""

import sys
from contextlib import ExitStack

for _p in ("/opt/trn_rl_repo", "/root/.axon_site/_ro/trn_rl_repo"):
    if _p not in sys.path:
        sys.path.append(_p)

import numpy as np

import concourse.bass as bass  # noqa: F401
import concourse.bacc as bacc
import concourse.mybir as mybir
import concourse.tile as tile
from concourse.bass import ts
from concourse.bass_utils import run_bass_kernel_spmd
from concourse.masks import make_identity, make_upper_triangular

FP32 = mybir.dt.float32
BF16 = mybir.dt.bfloat16

B, S, D, H, DH = 8, 1024, 768, 12, 64
P = 128
NS, NK = S // P, D // P  # 8 s-chunks, 6 k-tiles
NG = H // 2              # 6 head-pair groups
VW = DH + 1              # 65: V columns + ones column
N_CORES = 8


def _build_tile_kernel(tc, outs, ins):
    nc = tc.nc
    x, Wq, Wk, Wv = ins["x"], ins["Wq"], ins["Wk"], ins["Wv"]
    out = outs["out"]

    x_t = x.rearrange("(ns p) d -> p ns d", p=P)
    out_t = out.rearrange("(ns p) d -> p ns d", p=P)

    ctx = ExitStack()
    with ctx:
        consts = ctx.enter_context(tc.tile_pool(name="consts", bufs=1))
        sb1 = ctx.enter_context(tc.tile_pool(name="sb1", bufs=1))
        win = ctx.enter_context(tc.tile_pool(name="win", bufs=4))
        xin = ctx.enter_context(tc.tile_pool(name="xin", bufs=8))
        ptp = ctx.enter_context(tc.tile_pool(name="ptp", bufs=6))
        ctxs = ctx.enter_context(tc.tile_pool(name="ctxs", bufs=2))
        recp = ctx.enter_context(tc.tile_pool(name="recp", bufs=4))
        # PSUM: tr 2x1 + sc 2x2 + ctx 1x2 = 8 banks exactly.
        ps_tr = ctx.enter_context(tc.tile_pool(name="ps_tr", bufs=2, space="PSUM"))
        ps_sc = ctx.enter_context(tc.tile_pool(name="ps_sc", bufs=2, space="PSUM"))
        ps_ctx = ctx.enter_context(tc.tile_pool(name="ps_ctx", bufs=1, space="PSUM"))

        ident = consts.tile([P, P], FP32)
        make_identity(nc, ident)
        maskT = consts.tile([P, P], BF16)
        make_upper_triangular(nc, maskT, val=1.0, diag=True)

        xT = sb1.tile([P, NK, S], BF16)
        Wq_sb = sb1.tile([P, NK // 2, 2, H, DH], BF16)
        Wk_sb = sb1.tile([P, NK // 2, 2, H, DH], BF16)
        Wv_sb = sb1.tile([P, NK // 2, 2, H, DH], BF16)
        QT = sb1.tile([P, NG, S], BF16)
        KT = sb1.tile([P, NG, S], BF16)
        Vp = sb1.tile([P, NS, H * VW], BF16)
        out_sb = sb1.tile([P, NS, D], FP32)

        nc.gpsimd.memset(
            Vp.rearrange("p ns (h w) -> p ns h w", w=VW)[:, :, :, DH:VW], 1.0
        )

        def load_w_chunk(w_dram, w_sb, kt2, h0, h1):
            # Two consecutive D-rows per partition line: 512B-contiguous on
            # both DMA sides (full SDMA rate; <512B runs pay a 2x penalty).
            # Contraction K-tile (kt2, two) maps partition p to D-row
            # kt2*256 + 2p + two; x^T uses the same permuted order.
            nh = h1 - h0
            wtmp = win.tile([P, H // 2, 2 * DH], FP32, tag="w")
            # W DMAs ride the ACT HWDGE ring (x rides the SP ring)
            nc.scalar.dma_start(
                out=wtmp[:, 0:nh, :],
                in_=w_dram[h0:h1, kt2 * 256 : (kt2 + 1) * 256, :].rearrange(
                    "h (p two) d -> p h (two d)", two=2
                ),
            )
            # f32 -> bf16 cast (Wv on DVE — fast, needed first for V proj;
            # Wq/Wk alternate Pool / DVE); also reshuffles to [kt2, two, h, d]
            # so matmul slices for a K-tile (kt2, two) are contiguous.
            if w_dram is Wv:
                eng = nc.vector
            else:
                eng = nc.gpsimd if (kt2 % 2 == 0) else nc.vector
            eng.tensor_copy(
                out=w_sb[:, kt2, :, h0:h1, :],
                in_=wtmp[:, 0:nh, :].rearrange("p h (two d) -> p two h d", two=2),
            )

        # Moderately sized W DMAs (per-DMA HWDGE overhead is ~0.6us), in
        # availability order: all three Wv K-tiles first (V proj runs first),
        # then Wq/Wk, first-half heads before second-half. x chunks ride the
        # SP ring, W the ACT ring, so descriptor generation overlaps.
        xcs = []
        for ns in range(NS):
            xc = xin.tile([P, D], FP32, tag="xc")
            nc.sync.dma_start(out=xc, in_=x_t[:, ns, :])
            xcs.append(xc)
        w_order = [(Wv, Wv_sb, kt2) for kt2 in range(3)] + [
            (w, w_sb, kt2)
            for kt2 in range(3)
            for w, w_sb in ((Wq, Wq_sb), (Wk, Wk_sb))
        ]
        for w_dram, w_sb, kt2 in w_order:
            load_w_chunk(w_dram, w_sb, kt2, 0, 6)
        for w_dram, w_sb, kt2 in w_order:
            load_w_chunk(w_dram, w_sb, kt2, 6, 12)

        # x transposes (permuted-D order to match the W layout), batched 3
        # per PSUM bank so each batch needs only one DVE copy.
        for ns in range(NS):
            xcv = xcs[ns].rearrange("p (kt2 q two) -> p kt2 two q", kt2=3, two=2)
            for half in range(2):
                xtp = ps_tr.tile([P, 512], FP32, tag="tr", name="xtp")
                for i in range(3):
                    kt = half * 3 + i
                    kt2, two = divmod(kt, 2)
                    nc.tensor.transpose(
                        xtp[:, i * P : (i + 1) * P], xcv[:, kt2, two, :], ident
                    )
                nc.vector.tensor_copy(
                    xT[:, half * 3 : half * 3 + 3, ts(ns, P)],
                    xtp[:, 0 : 3 * P].rearrange("p (k q) -> p k q", k=3),
                )

        # ---- emission units for the software-pipelined main loop ----

        def vproj_unit(hf, ns):
            # half hf covers heads 6*hf .. 6*hf+5 (384 columns, one PSUM bank);
            # stationary x^T block reused across both halves' matmuls by the
            # caller pairing (same kt order).
            def emit():
                accv = ps_sc.tile([P, 1024], FP32, tag="sc", name="accv")
                for kt in range(NK):
                    kt2, two = divmod(kt, 2)
                    nc.tensor.matmul(
                        accv[:, 0:384],
                        xT[:, kt, ts(ns, P)],
                        Wv_sb[:, kt2, two, 6 * hf : 6 * hf + 6, :],
                        start=(kt == 0),
                        stop=(kt == NK - 1),
                    )
                nc.vector.tensor_copy(
                    Vp.rearrange("p ns (h w) -> p ns h w", w=VW)[
                        :, ns, 6 * hf : 6 * hf + 6, 0:DH
                    ],
                    accv[:, 0:384].rearrange("p (h d) -> p h d", d=DH),
                )

            return emit

        def qkproj_unit(g, w_sb, dstT):
            # accumulators live on the tr bank slots so the score-tile ("sc")
            # rotation never blocks on a projection copy
            def emit():
                acc0 = ps_tr.tile([P, 512], FP32, tag="tr", name="acc0")
                acc1 = ps_tr.tile([P, 512], FP32, tag="tr", name="acc1")
                for kt in range(NK):
                    kt2, two = divmod(kt, 2)
                    for c, acc in ((0, acc0), (1, acc1)):
                        nc.tensor.matmul(
                            acc[:, 0:512],
                            w_sb[:, kt2, two, 2 * g : 2 * g + 2, :],
                            xT[:, kt, ts(c, 512)],
                            start=(kt == 0),
                            stop=(kt == NK - 1),
                        )
                for c, acc in ((0, acc0), (1, acc1)):
                    nc.vector.tensor_copy(dstT[:, g, ts(c, 512)], acc[:, 0:512])

            return emit

        def proj_units(g):
            units = []
            if g == 0:
                units += [vproj_unit(0, ns) for ns in range(NS)]
            elif g == 3:
                units += [vproj_unit(1, ns) for ns in range(NS)]
            for w_sb, dstT in ((Wq_sb, QT), (Wk_sb, KT)):
                units.append(qkproj_unit(g, w_sb, dstT))
            return units

        def attention_head_units(h):
            """Returns (stream_units, norm_units).

            stream = score/exp units skewed one step ahead of PV units so
            PE's in-order queue always has independent score matmuls to chew
            on while ScalarE exp + GpSimd mask of the previous chunk drain.
            norm_units are emitted by the caller one head later.
            """
            po = (h % 2) * DH
            g = h // 2
            state = {}

            def smm_unit(j):
                def emit():
                    s0 = j * P
                    sext = S - s0
                    ptile = ptp.tile([P, S], BF16, tag="pt", name="ptile")
                    state["pt%d" % j] = ptile
                    sc = ps_sc.tile([P, 1024], FP32, tag="sc", name="scs")
                    for c in range((sext + 511) // 512):
                        cw = min(512, sext - c * 512)
                        nc.tensor.matmul(
                            sc[:, c * 512 : c * 512 + cw],
                            KT[po : po + DH, g, ts(j, P)],
                            QT[po : po + DH, g, s0 + c * 512 : s0 + c * 512 + cw],
                            start=True,
                            stop=True,
                        )
                    # one exp per key-chunk (reads both PSUM banks of sc)
                    nc.scalar.activation(
                        out=ptile[:, 0:sext],
                        in_=sc[:, 0:sext],
                        func=mybir.ActivationFunctionType.Exp,
                        scale=0.125,
                    )
                    # causal mask on the diagonal block (GpSimd: SBUF-only op)
                    nc.gpsimd.tensor_mul(ptile[:, 0:P], ptile[:, 0:P], maskT)

                return emit

            def pv_unit(j):
                def emit():
                    if j == 0:
                        state["ctx"] = ps_ctx.tile(
                            [VW, S], FP32, tag="ctx", name="ctxps"
                        )
                    s0 = j * P
                    ptile = state["pt%d" % j]
                    bounds = sorted({b for b in (s0, 512, S) if s0 <= b <= S})
                    # reverse order: the mask-gated diagonal matmul goes last
                    for b0, b1 in reversed(list(zip(bounds[:-1], bounds[1:]))):
                        nc.tensor.matmul(
                            state["ctx"][:, b0:b1],
                            Vp[:, j, h * VW : (h + 1) * VW],
                            ptile[:, b0 - s0 : b1 - s0],
                            start=(j == 0),
                            stop=(j == NS - 1),
                            skip_group_check=True,
                        )

                return emit

            def ctx_copy_unit():
                def emit():
                    ctx_sb = ctxs.tile([VW, S], FP32, tag="ctxs", name="ctxsb")
                    for c in range(2):
                        nc.vector.tensor_copy(
                            ctx_sb[:, ts(c, 512)], state["ctx"][:, ts(c, 512)]
                        )
                    state["ctx_sb"] = ctx_sb

                return emit

            def norm_unit(m0):
                def emit():
                    # 4 transposed s-chunks into one PSUM bank, then one
                    # strided reciprocal + one broadcast multiply.
                    trm = ps_tr.tile([P, 512], FP32, tag="tr", name="trm")
                    for i in range(4):
                        nc.tensor.transpose(
                            trm[:, i * VW : (i + 1) * VW],
                            state["ctx_sb"][:, ts(m0 + i, P)],
                            ident[0:VW, 0:VW],
                        )
                    trv = trm[:, 0 : 4 * VW].rearrange("p (m w) -> p m w", w=VW)
                    rec = recp.tile([P, 4, 1], FP32, tag="rec")
                    nc.vector.reciprocal(rec, trv[:, :, DH:VW])
                    nc.vector.tensor_mul(
                        out_sb[:, m0 : m0 + 4, h * DH : (h + 1) * DH],
                        trv[:, :, 0:DH],
                        rec.broadcast_to([P, 4, DH]),
                    )

                return emit

            stream = [smm_unit(0), smm_unit(1)]
            for j in range(2, NS):
                stream += [pv_unit(j - 2), smm_unit(j)]
            stream += [pv_unit(NS - 2), pv_unit(NS - 1), ctx_copy_unit()]
            return stream, [norm_unit(0), norm_unit(4)]

        # Software pipeline: group g's projections emit interleaved with
        # group g-1's attention so ScalarE exp always overlaps PE matmuls;
        # each head's normalization is deferred into the next head's stream.
        pending_norms = []
        for gi in range(NG + 1):
            att = []
            if gi >= 1:
                for h in (2 * (gi - 1), 2 * gi - 1):
                    stream, norms = attention_head_units(h)
                    # fold the previous head's norm units into this stream
                    for k, nu in enumerate(pending_norms):
                        stream.insert(4 + 5 * k, nu)
                    att += stream
                    pending_norms = norms
            prj = proj_units(gi) if gi < NG else []
            # proportional round-robin merge
            na, np_ = len(att), len(prj)
            ia = ip = 0
            while ia < na or ip < np_:
                if ip * max(na, 1) <= ia * max(np_, 1):
                    if ip < np_:
                        prj[ip]()
                        ip += 1
                    else:
                        att[ia]()
                        ia += 1
                else:
                    if ia < na:
                        att[ia]()
                        ia += 1
                    else:
                        prj[ip]()
                        ip += 1
        for nu in pending_norms:
            nu()

        # output DMAs spread over three HWDGE rings (all idle by now) so the
        # final flush isn't serialized on one ring's descriptor generation
        rings = (nc.sync, nc.scalar, nc.gpsimd)
        di = 0
        for c0 in (0, 6 * DH):
            for ns in range(NS):
                rings[di % 3].dma_start(
                    out=out_t[:, ns, c0 : c0 + 6 * DH],
                    in_=out_sb[:, ns, c0 : c0 + 6 * DH],
                )
                di += 1


_NC = {}


def build_nc(reps=1):
    """Build + compile the per-core Bass program once per process.

    reps > 1 emits the body multiple times with all-engine barriers between
    repetitions — used only for marginal-time measurement in test harnesses.
    """
    if reps in _NC:
        return _NC[reps]
    nc = bacc.Bacc("TRN2", target_bir_lowering=False, debug=False)
    ins = {
        "x": nc.dram_tensor("x", [S, D], FP32, kind="ExternalInput").ap(),
        "Wq": nc.dram_tensor("Wq", [H, D, DH], FP32, kind="ExternalInput").ap(),
        "Wk": nc.dram_tensor("Wk", [H, D, DH], FP32, kind="ExternalInput").ap(),
        "Wv": nc.dram_tensor("Wv", [H, D, DH], FP32, kind="ExternalInput").ap(),
    }
    outs = {"out": nc.dram_tensor("out", [S, D], FP32, kind="ExternalOutput").ap()}
    with tile.TileContext(nc) as tc:
        for i in range(reps):
            if i:
                tc.strict_bb_all_engine_barrier()
            _build_tile_kernel(tc, outs, ins)
    nc.compile()
    _NC[reps] = nc
    return nc


def make_in_maps(x, Wq, Wk, Wv):
    x = np.ascontiguousarray(x, dtype=np.float32)
    Wq = np.ascontiguousarray(Wq, dtype=np.float32)
    Wk = np.ascontiguousarray(Wk, dtype=np.float32)
    Wv = np.ascontiguousarray(Wv, dtype=np.float32)
    return [
        {"x": np.ascontiguousarray(x[b]), "Wq": Wq, "Wk": Wk, "Wv": Wv}
        for b in range(B)
    ]


def kernel(x, Wq, Wk, Wv):
    nc = build_nc()
    res = run_bass_kernel_spmd(nc, make_in_maps(x, Wq, Wk, Wv), list(range(N_CORES)))
    return np.stack([res.results[b]["out"] for b in range(B)], axis=0)
